# revision 1
# baseline (speedup 1.0000x reference)
"""Trainium2 Bass kernel for nn_CrossModalAttentionModule.

Math restructuring: the reference output is v2s[b,s] = sum_c final[b,s,c]*V_final[s,c]
with final = mean_n(feat) + RATIO*(softmax_n(query@k^T) @ v) @ Wo^T. Because the
output contracts everything against V_final, the M=1024-dim attention collapses:

  scores[b,s,n] = sum_c Qk[s,c] * feat[b,c,n]        Qk  = (att_emb@Wq^T+bq) @ Wk
  W[b,s,n]      = sum_c Wv2[s,c] * feat[b,c,n]       Wv2 = (RATIO*V_final@Wo) @ Wv
  v2s[b,s]      = mean_n(feat[b])@V_final[s] + softmax_n(scores)·W + const[s]

409 GFLOP -> 64 GFLOP. Data-parallel over batch: 16 batches per core on 8 cores.

Precision: fp16 operands at full PE rate (fp32/fp32r matmul is 4x slower on TRN2).
Scores feed a softmax whose near-ties amplify input noise, so scores use a 3-pass
fp16 error-compensated product (fh*qh + fh*ql + fl*qh accumulated in one PSUM
tile in fp32), which is fp32-grade. The linear W term tolerates plain fp16.
"""

import os
import sys
import types
from contextlib import ExitStack

sys.path.insert(0, "/opt/trn_rl_repo")

import numpy as np

import concourse.bass as bass
import concourse.mybir as mybir
import concourse.tile as tile
from concourse import bacc, bass_utils
from concourse.bass import ts

# Optional NTFF profiling hook (used when BASS_TRACE=1); missing module on this image.
try:
    import antenv.axon_hooks  # noqa: F401
except ImportError:
    try:
        import trn_agent_boot.trn_boot as _tb

        _hook = _tb._ntff_profile_via_ctypes("/opt/axon/libaxon_pjrt.so")
        _m = types.ModuleType("antenv.axon_hooks")
        _m.get_axon_ntff_profile_hook = lambda: _hook
        _m.set_axon_ntff_profile_hook = lambda h: None
        sys.modules["antenv.axon_hooks"] = _m
        import antenv

        antenv.axon_hooks = _m
    except Exception:
        pass

F32 = mybir.dt.float32
F16 = mybir.dt.float16

RATIO = 1.0
B, C, N = 128, 2048, 196
S, L, M = 312, 300, 1024
NCORES = 8
BPC = B // NCORES          # batches per core: 16
NPAIR = BPC // 2           # batch pairs per core: 8
KCH = C // 128             # contraction chunks: 16
SCH = 3                    # S row chunks (312 -> 3x128 with zero pad)
SPAD = SCH * 128           # 384
W2 = 2 * N                 # 392 moving cols (2 batches)

_CACHE = {}


def _build(n_score_passes=3):
    nc = bacc.Bacc("TRN2", target_bir_lowering=False, debug=False, num_devices=NCORES)

    fh_d = nc.dram_tensor("fh", [NPAIR, KCH, 128, W2], F16, kind="ExternalInput").ap()
    fl_d = nc.dram_tensor("fl", [NPAIR, KCH, 128, W2], F16, kind="ExternalInput").ap()
    # per k-chunk weight columns: [qh(384) | ql(384) | wh(384)]
    gt_d = nc.dram_tensor("gt", [KCH, 128, 3 * SPAD], F16, kind="ExternalInput").ap()
    vt_d = nc.dram_tensor("vt", [KCH, 128, SPAD], F16, kind="ExternalInput").ap()
    ct_d = nc.dram_tensor("ct", [128, SCH], F32, kind="ExternalInput").ap()
    out_d = nc.dram_tensor("out", [128, SCH * BPC], F32, kind="ExternalOutput").ap()

    with tile.TileContext(nc) as tc:
        with ExitStack() as ctx:
            wpool = ctx.enter_context(tc.tile_pool(name="wts", bufs=1))
            fpool = ctx.enter_context(tc.tile_pool(name="feat", bufs=2))
            epool = ctx.enter_context(tc.tile_pool(name="exp", bufs=3))
            tpool = ctx.enter_context(tc.tile_pool(name="trash", bufs=2))
            mpool = ctx.enter_context(tc.tile_pool(name="mx", bufs=4))
            apool = ctx.enter_context(tc.tile_pool(name="acc", bufs=1))
            ps = ctx.enter_context(tc.tile_pool(name="ps", bufs=3, space="PSUM"))
            pw = ctx.enter_context(tc.tile_pool(name="pw", bufs=2, space="PSUM"))
            pt = ctx.enter_context(tc.tile_pool(name="pt1", bufs=1, space="PSUM"))

            g_sb = wpool.tile([128, KCH * 3 * SPAD], F16, tag="g")
            vt_sb = wpool.tile([128, KCH * SPAD], F16, tag="vt")
            ct_sb = wpool.tile([128, SCH], F32, tag="ct")
            nc.sync.dma_start(g_sb[:].rearrange("p (k m) -> p k m", k=KCH),
                              gt_d.rearrange("k p m -> p k m"))
            nc.sync.dma_start(vt_sb[:].rearrange("p (k m) -> p k m", k=KCH),
                              vt_d.rearrange("k p m -> p k m"))
            nc.sync.dma_start(ct_sb[:], ct_d)

            # accumulators across pairs
            den = [apool.tile([128, BPC], F32, tag=f"den{i}", name=f"den{i}") for i in range(SCH)]
            num = [apool.tile([128, BPC], F32, tag=f"num{i}", name=f"num{i}") for i in range(SCH)]
            pool_h = apool.tile([128, KCH * BPC], F32, tag="poolh")
            pool_l = apool.tile([128, KCH * BPC], F32, tag="pooll")

            def gslice(k, blk, i):
                # weight block blk in {0:qh, 1:ql, 2:wh}, row chunk i
                base = k * 3 * SPAD + blk * SPAD + i * 128
                return g_sb[:, base:base + 128]

            for pair in range(NPAIR):
                fh_sb = fpool.tile([128, KCH * W2], F16, tag="fh")
                fl_sb = fpool.tile([128, KCH * W2], F16, tag="fl")
                nc.sync.dma_start(fh_sb[:].rearrange("p (k n) -> p k n", k=KCH),
                                  fh_d[pair].rearrange("k p n -> p k n"))
                if n_score_passes == 3:
                    nc.sync.dma_start(fl_sb[:].rearrange("p (k n) -> p k n", k=KCH),
                                      fl_d[pair].rearrange("k p n -> p k n"))

                # feat pooling: mean over n per batch (sum of fh and fl parts)
                fh3 = fh_sb[:].rearrange("p (k n) -> p k n", k=KCH)
                fl3 = fl_sb[:].rearrange("p (k n) -> p k n", k=KCH)
                ph3 = pool_h[:].rearrange("p (k b) -> p k b", k=KCH)
                pl3 = pool_l[:].rearrange("p (k b) -> p k b", k=KCH)
                for pb in range(2):
                    b = 2 * pair + pb
                    nc.vector.reduce_sum(ph3[:, :, b:b + 1], fh3[:, :, ts(pb, N)],
                                         axis=mybir.AxisListType.X)
                    if n_score_passes == 3:
                        nc.vector.reduce_sum(pl3[:, :, b:b + 1], fl3[:, :, ts(pb, N)],
                                             axis=mybir.AxisListType.X)

                for i in range(SCH):
                    ps_s = ps.tile([128, W2], F32, tag="ps")
                    first = True
                    for k in range(KCH):
                        nc.tensor.matmul(ps_s[:], gslice(k, 0, i), fh_sb[:, ts(k, W2)],
                                         start=first, stop=False)
                        first = False
                    if n_score_passes == 3:
                        for k in range(KCH):
                            nc.tensor.matmul(ps_s[:], gslice(k, 1, i), fh_sb[:, ts(k, W2)],
                                             start=False, stop=False)
                        for k in range(KCH):
                            nc.tensor.matmul(ps_s[:], gslice(k, 0, i), fl_sb[:, ts(k, W2)],
                                             start=False, stop=(k == KCH - 1))
                    ps_w = pw.tile([128, W2], F32, tag="pw")
                    for k in range(KCH):
                        nc.tensor.matmul(ps_w[:], gslice(k, 2, i), fh_sb[:, ts(k, W2)],
                                         start=(k == 0), stop=(k == KCH - 1))

                    # softmax over n (per batch segment) + dot with W
                    nmx = mpool.tile([128, 2], F32, tag="nmx")
                    nc.vector.reduce_max(nmx[:], ps_s[:].rearrange("p (t n) -> p t n", t=2),
                                         axis=mybir.AxisListType.X, negate=True)
                    et = epool.tile([128, W2], F32, tag="et")
                    tr = tpool.tile([128, W2], F32, tag="tr")
                    for pb in range(2):
                        b = 2 * pair + pb
                        nc.scalar.activation(et[:, ts(pb, N)], ps_s[:, ts(pb, N)],
                                             mybir.ActivationFunctionType.Exp,
                                             bias=nmx[:, pb:pb + 1], scale=1.0,
                                             accum_out=den[i][:, b:b + 1])
                        nc.vector.tensor_mul(tr[:, ts(pb, N)], et[:, ts(pb, N)],
                                             ps_w[:, ts(pb, N)])
                        nc.vector.reduce_sum(num[i][:, b:b + 1], tr[:, ts(pb, N)],
                                             axis=mybir.AxisListType.X)

            # t1 = mean_n(feat) @ V^T / N  via PE: cast pools to fp16, two halves in one rhs
            p16 = apool.tile([128, KCH * 32], F16, tag="p16")
            p163 = p16[:].rearrange("p (k b) -> p k b", k=KCH)
            nc.vector.tensor_copy(p163[:, :, 0:BPC], ph3[:, :, :])
            if n_score_passes == 3:
                nc.vector.tensor_copy(p163[:, :, BPC:2 * BPC], pl3[:, :, :])
            else:
                nc.gpsimd.memset(p16[:].rearrange("p (k b) -> p k b", k=KCH)[:, :, BPC:2 * BPC], 0.0)
            t1p_all = pt.tile([128, SCH * 32], F32, tag="t1", name="t1p_all")
            t1p = [t1p_all[:, 32 * i:32 * (i + 1)] for i in range(SCH)]
            for i in range(SCH):
                for k in range(KCH):
                    nc.tensor.matmul(t1p[i][:], vt_sb[:, k * SPAD + i * 128:k * SPAD + (i + 1) * 128], p16[:, ts(k, 32)],
                                     start=(k == 0), stop=(k == KCH - 1))

            # final: out = t1h + t1l + num/den + ct
            out_sb = wpool.tile([128, SCH * BPC], F32, tag="osb")
            for i in range(SCH):
                rden = mpool.tile([128, BPC], F32, tag="rden")
                nc.vector.reciprocal(rden[:], den[i][:])
                t2 = mpool.tile([128, BPC], F32, tag="t2")
                nc.vector.tensor_mul(t2[:], num[i][:], rden[:])
                t1s = mpool.tile([128, BPC], F32, tag="t1s")
                nc.vector.tensor_add(t1s[:], t2[:], t1p[i][:, 0:BPC])
                nc.vector.tensor_add(t1s[:], t1s[:], t1p[i][:, BPC:2 * BPC])
                nc.vector.tensor_scalar_add(out_sb[:, ts(i, BPC)], t1s[:], ct_sb[:, i:i + 1])
            nc.sync.dma_start(out_d, out_sb[:])

    nc.compile()
    return nc


def _prep(feat, att_emb, Wq, bq, Wk, bk, Wv, bv, Wo, bo, V_final):
    f64 = np.float64
    query = att_emb.astype(f64) @ Wq.T.astype(f64) + bq.astype(f64)   # [S, M]
    Qk = query @ Wk.astype(f64)                                        # [S, C]
    U = RATIO * (V_final.astype(f64) @ Wo.astype(f64))                 # [S, M]
    Wv2 = U @ Wv.astype(f64)                                           # [S, C]
    c1 = U @ bv.astype(f64)                                            # [S]
    c0 = RATIO * (V_final.astype(f64) @ bo.astype(f64))                # [S]
    cc = (c0 + c1).astype(np.float32)                                  # additive const

    def pack_w(mat):  # [S, C] -> fp16 [C, SPAD] high/low split
        mt = np.zeros((C, SPAD), np.float64)
        mt[:, :S] = mat.T
        hi = mt.astype(np.float16)
        lo = (mt - hi.astype(np.float64)).astype(np.float16)
        return hi, lo

    qh, ql = pack_w(Qk)
    wh, _ = pack_w(Wv2)
    gt = np.concatenate([qh.reshape(KCH, 128, SPAD), ql.reshape(KCH, 128, SPAD),
                         wh.reshape(KCH, 128, SPAD)], axis=2)          # [KCH,128,3*SPAD]

    vtp = np.zeros((C, SPAD), np.float64)
    vtp[:, :S] = V_final.T.astype(f64) / N
    vt = vtp.astype(np.float16).reshape(KCH, 128, SPAD)

    ct = np.zeros((128, SCH), np.float32)
    for i in range(SCH):
        lo_s, hi_s = i * 128, min((i + 1) * 128, S)
        ct[0:hi_s - lo_s, i] = cc[lo_s:hi_s]

    # feat -> fp16 split, packed [core, pair, k, p, 2*N]
    fh = feat.astype(np.float16)
    fl = (feat - fh.astype(np.float32)).astype(np.float16)

    def pack_f(a):  # [B, C, N] fp16 -> [NCORES, NPAIR, KCH, 128, W2]
        a = a.reshape(NCORES, NPAIR, 2, KCH, 128, N)
        return np.ascontiguousarray(a.transpose(0, 1, 3, 4, 2, 5)).reshape(
            NCORES, NPAIR, KCH, 128, W2)

    return pack_f(fh), pack_f(fl), gt, vt, ct


def kernel(feat, att_emb, Wq, bq, Wk, bk, Wv, bv, Wo, bo, V_final):
    n_passes = int(os.environ.get("XATTN_SCORE_PASSES", "3"))
    if "nc" not in _CACHE or _CACHE.get("passes") != n_passes:
        _CACHE["nc"] = _build(n_passes)
        _CACHE["passes"] = n_passes
    nc = _CACHE["nc"]

    fhp, flp, gt, vt, ct = _prep(feat.astype(np.float32), att_emb.astype(np.float32),
                                 Wq, bq, Wk, bk, Wv, bv, Wo, bo, V_final)
    in_maps = [
        {"fh": fhp[c], "fl": flp[c], "gt": gt, "vt": vt, "ct": ct}
        for c in range(NCORES)
    ]
    res = bass_utils.run_bass_kernel_spmd(
        nc, in_maps, core_ids=list(range(NCORES)),
        trace=bool(int(os.environ.get("XATTN_TRACE", "0"))))
    _CACHE["last_result"] = res

    out = np.empty((B, S), np.float32)
    for c in range(NCORES):
        o = res.results[c]["out"]                     # [128, SCH*BPC]
        for i in range(SCH):
            lo_s, hi_s = i * 128, min((i + 1) * 128, S)
            blk = o[0:hi_s - lo_s, i * BPC:(i + 1) * BPC]  # [rows, 16]
            out[c * BPC:(c + 1) * BPC, lo_s:hi_s] = blk.T
    return out


if __name__ == "__main__":
    rng = np.random.default_rng(1)
    inputs = {
        "feat": rng.standard_normal((B, C, N)).astype(np.float32),
        "att_emb": rng.standard_normal((S, L)).astype(np.float32),
        "Wq": (rng.standard_normal((M, L)) / np.sqrt(L)).astype(np.float32),
        "bq": np.zeros(M, np.float32),
        "Wk": (rng.standard_normal((M, C)) / np.sqrt(C)).astype(np.float32),
        "bk": np.zeros(M, np.float32),
        "Wv": (rng.standard_normal((M, C)) / np.sqrt(C)).astype(np.float32),
        "bv": np.zeros(M, np.float32),
        "Wo": (rng.standard_normal((C, M)) / np.sqrt(M)).astype(np.float32),
        "bo": np.zeros(C, np.float32),
        "V_final": rng.standard_normal((S, C)).astype(np.float32),
    }
    out = kernel(**inputs)
    print("out", out.shape, out.dtype, out.std())

